# revision 3
# baseline (speedup 1.0000x reference)
"""GSA block on 8 NeuronCores via Bass/Tile.

Sharding: core c -> batch b=c//2, head-pair hp=c%2 (heads 2hp, 2hp+1).
Each core receives deduplicated bf16 inputs (half of x_b per pair member,
a quarter of its head-pair weight bundle) which are completed on-device by
pair/4-group AllGathers. The device then runs: projections (bf16 matmuls),
a chunkwise GSA recurrence (C=128, 16 chunks, mid-centered gate cumsum for
bf16-safe exponent ranges), per-head RMSNorm fused from PSUM, and o @ Wo
partial products. A pair ReduceScatter sums the partial y and leaves each
core with half the rows, so every output byte crosses the (slow) axon
tunnel exactly once, in bf16.
"""
import os
import sys
import numpy as np

sys.path.insert(0, '/opt/trn_rl_repo')

B, T, D = 4, 2048, 1024
H, K, V, M = 4, 256, 256, 64
HP = 2                     # heads per core
KP = HP * K                # 512 projection cols per core (q/k/v)
MP = HP * M                # 128 gate cols per core
GATE_NORM = 8.0
EPS = 1e-5
SCALE = K ** -0.5
C = 128                    # chunk length
NCH = T // C               # 16 chunks
NKT = D // 128             # 8 contraction tiles over D
DT_T = 512                 # free-dim tile for projections
OFF_WQ = 0                 # weight-bundle element offsets (bf16 flat)
OFF_WK = D * KP
OFF_WV = 2 * D * KP
OFF_WF = 3 * D * KP
OFF_WO = 3 * D * KP + D * MP
WB = 3 * D * KP + D * MP + KP * D
WBQ = WB // 4              # per-core quarter of the bundle

_cache = {}
IN_ORDER = ("xh", "wb", "cn")      # must match DRAM declaration order
OUT_SHAPES = ((T // 2, D),)        # y


def _build_consts():
    j = np.arange(C)[:, None]
    c = np.arange(C)[None, :]
    mask_ut = (c >= j).astype(np.float32)          # [j, c] keep j<=c
    lt2 = mask_ut - (j < C // 2)                   # centered cumsum matrix
    eye = np.eye(C, dtype=np.float32)
    half_col = (j < C // 2).astype(np.float32)     # [C, 1]
    return np.concatenate([mask_ut, lt2, eye, half_col.reshape(C, 1)], axis=1)


def _build_program():
    import concourse.bacc as bacc
    import concourse.tile as tile
    import concourse.mybir as mybir

    f32, f32r, bf16 = mybir.dt.float32, mybir.dt.float32r, mybir.dt.bfloat16
    AF = mybir.ActivationFunctionType
    ALU = mybir.AluOpType
    AX = mybir.AxisListType

    nc = bacc.Bacc("TRN2", target_bir_lowering=False, debug=False,
                   num_devices=8)

    xh_d = nc.dram_tensor("xh", [D, T // 2], bf16, kind="ExternalInput").ap()
    wb_d = nc.dram_tensor("wb", [WBQ], bf16, kind="ExternalInput").ap()
    cn_d = nc.dram_tensor("cn", [C, 3 * C + 1], f32, kind="ExternalInput").ap()
    y_d = nc.dram_tensor("y", [T // 2, D], bf16, kind="ExternalOutput").ap()

    with tile.TileContext(nc) as tc:
        with (
            tc.tile_pool(name="dram", bufs=1, space="DRAM") as dram,
            tc.tile_pool(name="persist", bufs=1) as pp,
        ):
            SIM = bool(int(os.environ.get("BASS_TL_SIM", "0")))
            gxi = dram.tile([D, T // 2], bf16)
            gxo = dram.tile([2 * D, T // 2], bf16)
            nc.gpsimd.dma_start(gxi[:], xh_d[:])
            gwi = dram.tile([WBQ], bf16)
            gwo = dram.tile([WB], bf16)
            nc.gpsimd.dma_start(gwi[:], wb_d[:])
            if SIM:
                for r in range(2):
                    nc.gpsimd.dma_start(gxo[r * D:(r + 1) * D, :], gxi[:])
                for r in range(4):
                    nc.gpsimd.dma_start(gwo[r * WBQ:(r + 1) * WBQ], gwi[:])
            else:
                nc.gpsimd.collective_compute(
                    "AllGather", mybir.AluOpType.bypass,
                    replica_groups=[[0, 1], [2, 3], [4, 5], [6, 7]],
                    ins=[gxi.opt()], outs=[gxo.opt()])
                nc.gpsimd.collective_compute(
                    "AllGather", mybir.AluOpType.bypass,
                    replica_groups=[[0, 2, 4, 6], [1, 3, 5, 7]],
                    ins=[gwi.opt()], outs=[gwo.opt()])

            def wb_view(off, kt, rows, cols):
                seg = gwo[off + kt * rows * cols:off + (kt + 1) * rows * cols]
                return seg.rearrange("(p n) -> p n", p=rows)

            cn = pp.tile([C, 3 * C + 1], f32)
            nc.sync.dma_start(cn[:], cn_d[:])
            mask_ut = cn[:, 0:C]
            lt2 = cn[:, C:2 * C]
            eye = cn[:, 2 * C:3 * C]
            half_col = cn[:, 3 * C:3 * C + 1]
            ones1 = cn[0:1, 0:C]              # row 0 of mask_ut = ones
            eye_bf = pp.tile([C, C], bf16)
            nc.any.tensor_copy(eye_bf[:], cn[:, 2 * C:3 * C])
            epsc = pp.tile([C, 1], f32)
            nc.vector.memset(epsc[:], EPS)
            onec = pp.tile([C, 1], f32)
            nc.vector.memset(onec[:], 1.0)

            # ---- persistent activation storage ----
            qT = pp.tile([128, 4 * T], bf16)   # 4 col-tiles of [128, T]
            kT = pp.tile([128, 4 * T], bf16)
            vn = pp.tile([128, NCH * KP], bf16)  # 16 t-tiles of [128, 512]
            fT = pp.tile([128, T], f32)
            oT = pp.tile([128, 4 * T], bf16)
            wo = pp.tile([128, 4 * D], bf16)   # 4 kt tiles of [128, 1024]
            for kt in range(4):
                nc.sync.dma_start(wo[:, kt * D:(kt + 1) * D],
                                  wb_view(OFF_WO, kt, 128, D))

            with (
                tc.tile_pool(name="proj", bufs=1) as jp,
                tc.tile_pool(name="proj_ps", bufs=8, space="PSUM") as jps,
            ):
                xT = jp.tile([128, NKT * T], bf16)     # 8 kt tiles of [128, T]
                wq = jp.tile([128, NKT * KP], bf16)
                wk = jp.tile([128, NKT * KP], bf16)
                wv = jp.tile([128, NKT * KP], bf16)
                wf = jp.tile([128, NKT * MP], bf16)
                for kt in range(NKT):
                    rs = slice(kt * 128, (kt + 1) * 128)
                    for hf in range(2):
                        nc.sync.dma_start(
                            xT[:, kt * T + hf * (T // 2):
                               kt * T + (hf + 1) * (T // 2)],
                            gxo[hf * D + kt * 128:hf * D + (kt + 1) * 128, :])
                    nc.sync.dma_start(wq[:, kt * KP:(kt + 1) * KP],
                                      wb_view(OFF_WQ, kt, 128, KP))
                    nc.sync.dma_start(wk[:, kt * KP:(kt + 1) * KP],
                                      wb_view(OFF_WK, kt, 128, KP))
                    nc.sync.dma_start(wv[:, kt * KP:(kt + 1) * KP],
                                      wb_view(OFF_WV, kt, 128, KP))
                    nc.sync.dma_start(wf[:, kt * MP:(kt + 1) * MP],
                                      wb_view(OFF_WF, kt, 128, MP))

                def xTb(kt, t0, ts):
                    return xT[:, kt * T + t0:kt * T + t0 + ts]

                # qT/kT: [KP, T] transposed outputs; 4 col-tiles x 4 t-tiles
                for w_sb, outT, act in ((wq, qT, AF.Silu), (wk, kT, AF.Silu)):
                    for cc in range(4):           # out partition tile (q cols)
                        for tt in range(4):       # free t tile
                            ps = jps.tile([128, DT_T], f32, tag="pjps")
                            for kt in range(NKT):
                                lhs = w_sb[:, kt * KP + cc * 128:
                                           kt * KP + (cc + 1) * 128]
                                nc.tensor.matmul(
                                    ps[:], lhs,
                                    xTb(kt, tt * DT_T, DT_T),
                                    start=(kt == 0), stop=(kt == NKT - 1))
                            nc.scalar.activation(
                                outT[:, cc * T + tt * DT_T:
                                     cc * T + (tt + 1) * DT_T], ps[:], act)
                # v natural: [T, KP]; 16 t-tiles [128, 512]
                for tt in range(NCH):
                    ps = jps.tile([128, KP], f32, tag="pjps")
                    for kt in range(NKT):
                        lhs = xT[:, kt * T + tt * 128:kt * T + (tt + 1) * 128]
                        nc.tensor.matmul(
                            ps[:], lhs,
                            wv[:, kt * KP:(kt + 1) * KP],
                            start=(kt == 0), stop=(kt == NKT - 1))
                    nc.any.tensor_copy(vn[:, tt * KP:(tt + 1) * KP], ps[:])
                # fT: [MP, T] transposed; raw -> logsigmoid/GATE_NORM
                for tt in range(4):
                    ps = jps.tile([128, DT_T], f32, tag="pjps")
                    for kt in range(NKT):
                        lhs = wf[:, kt * MP:(kt + 1) * MP]
                        nc.tensor.matmul(
                            ps[:], lhs,
                            xTb(kt, tt * DT_T, DT_T),
                            start=(kt == 0), stop=(kt == NKT - 1))
                    sp = jp.tile([128, DT_T], f32, tag="fsp")
                    nc.scalar.activation(sp[:], ps[:], AF.Exp, scale=-1.0)
                    sp2 = jp.tile([128, DT_T], f32, tag="fsp2")
                    nc.scalar.activation(sp2[:], sp[:], AF.Ln, bias=onec[:])
                    nc.vector.tensor_scalar_mul(
                        fT[:, tt * DT_T:(tt + 1) * DT_T], sp2[:],
                        -1.0 / GATE_NORM)

            # ---- recurrence ----
            with (
                tc.tile_pool(name="st", bufs=2) as stp,
                tc.tile_pool(name="ck", bufs=3) as ckp,
                tc.tile_pool(name="ck1", bufs=3) as ck1,
                tc.tile_pool(name="ps_g", bufs=2, space="PSUM") as psg,
                tc.tile_pool(name="ps_tp", bufs=2, space="PSUM") as pst,
                tc.tile_pool(name="ps_gr", bufs=2, space="PSUM") as psgr,
                tc.tile_pool(name="ps_big", bufs=2, space="PSUM") as psbig,
            ):
                yb = dram.tile([T, D], bf16)
                ybh = dram.tile([T // 2, D], bf16)
                state0 = stp.tile([128, 512], f32, tag="state")
                nc.vector.memset(state0[:], 0.0)
                state = state0
                for i in range(NCH):
                    t0 = i * C
                    # gates
                    fnP = psg.tile([C, 128], f32, tag="g")
                    nc.tensor.transpose(fnP[:], fT[:, t0:t0 + C], eye)
                    fn = ckp.tile([C, 128], f32, tag="fn")
                    nc.any.tensor_copy(fn[:], fnP[:])
                    cumP = psg.tile([C, 128], f32, tag="g")
                    nc.tensor.matmul(cumP[:], lt2, fn[:])
                    midP = psg.tile([1, 128], f32, tag="g")
                    nc.tensor.matmul(midP[:], half_col, fn[:])
                    ai = ckp.tile([C, 128], f32, tag="ai")
                    nc.scalar.activation(ai[:], cumP[:], AF.Exp)
                    ain = ckp.tile([C, 128], f32, tag="ain")
                    nc.scalar.activation(ain[:], cumP[:], AF.Exp, scale=-1.0)
                    emid = ckp.tile([1, 128], f32, tag="emid")
                    nc.scalar.activation(emid[:], midP[:], AF.Exp)
                    ef = ckp.tile([C, 128], f32, tag="ef")
                    nc.scalar.activation(ef[:], fn[:], AF.Exp)
                    sn = ckp.tile([C, 128], f32, tag="sn")
                    nc.vector.tensor_scalar(sn[:], ef[:], -1.0, 1.0,
                                            op0=ALU.mult, op1=ALU.add)
                    stil = ckp.tile([C, 128], bf16, tag="stil")
                    nc.vector.tensor_mul(stil[:], sn[:], ain[:])
                    alP = psg.tile([1, 128], f32, tag="g")
                    nc.tensor.matmul(alP[:], lt2[:, C - 1:C], fn[:])
                    ail = ckp.tile([1, 128], f32, tag="ail")
                    nc.scalar.activation(ail[:], alP[:], AF.Exp)
                    atot = ckp.tile([1, 128], f32, tag="atot")
                    nc.vector.tensor_mul(atot[:], ail[:], emid[:])
                    bcP = psg.tile([C, 128], f32, tag="g")
                    nc.tensor.matmul(bcP[:], ones1, ail[:])
                    sa = ckp.tile([C, 128], bf16, tag="sa")
                    nc.vector.tensor_mul(sa[:], stil[:], bcP[:])
                    atcP = psg.tile([128, 1], f32, tag="g")
                    nc.tensor.transpose(atcP[:], atot[:], eye[0:1, 0:1])
                    atc = ckp.tile([128, 1], f32, tag="atc")
                    nc.any.tensor_copy(atc[:], atcP[:])
                    emcP = psg.tile([128, 1], f32, tag="g")
                    nc.tensor.transpose(emcP[:], emid[:], eye[0:1, 0:1])
                    emc = ckp.tile([128, 1], f32, tag="emc")
                    nc.any.tensor_copy(emc[:], emcP[:])
                    # scaled states (hkT*emid -> transpose; hv*emid)
                    hkTs = ck1.tile([128, 256], bf16, tag="hkTs")
                    nc.vector.tensor_scalar_mul(hkTs[:], state[:, 0:256],
                                                emc[:])
                    hv = ck1.tile([128, 256], bf16, tag="hv")
                    nc.vector.tensor_scalar_mul(hv[:], state[:, 256:512],
                                                emc[:])
                    hkn = ck1.tile([128, 256], bf16, tag="hkn")
                    for kt in range(2):
                        hknP = pst.tile([128, 128], bf16, tag="tp")
                        nc.tensor.transpose(
                            hknP[:], hkTs[:, kt * 128:(kt + 1) * 128], eye_bf)
                        nc.any.tensor_copy(
                            hkn[:, kt * 128:(kt + 1) * 128], hknP[:])
                    # k natural from kT transposes: [C, (h,kt) 128] x4
                    knat = ck1.tile([128, 512], bf16, tag="knat")
                    for h in range(HP):
                        for kt in range(2):
                            cc = 2 * h + kt
                            knP = pst.tile([128, 128], bf16, tag="tp")
                            nc.tensor.transpose(
                                knP[:], kT[:, cc * T + t0:cc * T + t0 + C],
                                eye_bf)
                            nc.any.tensor_copy(
                                knat[:, h * 256 + kt * 128:
                                     h * 256 + (kt + 1) * 128], knP[:])
                    # KQ grams + logits
                    lgP = psgr.tile([C, 128], f32, tag="gram")
                    kqm = ck1.tile([C, 2 * C], bf16, tag="kqm")
                    for h in range(HP):
                        kqP = psgr.tile([C, C], f32, tag="gram")
                        for kt in range(2):
                            cc = 2 * h + kt
                            nc.tensor.matmul(
                                kqP[:], kT[:, cc * T + t0:cc * T + t0 + C],
                                qT[:, cc * T + t0:cc * T + t0 + C],
                                start=(kt == 0), stop=(kt == 1))
                        nc.vector.tensor_mul(
                            kqm[:, h * C:(h + 1) * C], kqP[:], mask_ut)
                        for kt in range(2):
                            cc = 2 * h + kt
                            nc.tensor.matmul(
                                lgP[:, h * M:(h + 1) * M],
                                qT[:, cc * T + t0:cc * T + t0 + C],
                                hkn[:, kt * 128 + h * M:
                                    kt * 128 + (h + 1) * M],
                                start=(kt == 0), stop=False)
                        nc.tensor.matmul(
                            lgP[:, h * M:(h + 1) * M],
                            kqm[:, h * C:(h + 1) * C],
                            stil[:, h * M:(h + 1) * M],
                            start=False, stop=True)
                    lg = ckp.tile([C, 128], f32, tag="lg")
                    nc.vector.tensor_mul(lg[:], lgP[:], ai[:])
                    # softmax over M per head; pt = p * ai
                    pt = ckp.tile([C, 128], bf16, tag="pt")
                    for h in range(HP):
                        hs = slice(h * M, (h + 1) * M)
                        nmax = ckp.tile([C, 1], f32, tag="nmax")
                        nc.vector.tensor_reduce(nmax[:], lg[:, hs], axis=AX.X,
                                                op=ALU.max, negate=True)
                        bias = ckp.tile([C, 1], f32, tag="bias")
                        nc.vector.tensor_scalar_mul(bias[:], nmax[:], SCALE)
                        e = ckp.tile([C, M], f32, tag="e")
                        esum = ckp.tile([C, 1], f32, tag="esum")
                        nc.scalar.activation(e[:], lg[:, hs], AF.Exp,
                                             bias=bias[:], scale=SCALE,
                                             accum_out=esum[:])
                        rcp = ckp.tile([C, 1], f32, tag="rcp")
                        nc.vector.reciprocal(rcp[:], esum[:])
                        p1 = ckp.tile([C, M], f32, tag="p1")
                        nc.vector.tensor_scalar_mul(p1[:], e[:], rcp[:])
                        nc.vector.tensor_mul(pt[:, hs], p1[:], ai[:, hs])
                    ptT = ck1.tile([128, C], bf16, tag="ptT")
                    ptTP = pst.tile([128, C], bf16, tag="tp")
                    nc.tensor.transpose(ptTP[:], pt[:], eye_bf)
                    nc.any.tensor_copy(ptT[:], ptTP[:])
                    stT = ck1.tile([128, C], bf16, tag="stT")
                    stTP = pst.tile([128, C], bf16, tag="tp")
                    nc.tensor.transpose(stTP[:], stil[:], eye_bf)
                    nc.any.tensor_copy(stT[:], stTP[:])
                    # o = pt @ hv + tril(PS) @ v
                    oP = psbig.tile([C, 512], f32, tag="big")
                    for h in range(HP):
                        psP = psgr.tile([C, C], f32, tag="gram")
                        nc.tensor.matmul(psP[:], stT[h * M:(h + 1) * M, :],
                                         ptT[h * M:(h + 1) * M, :])
                        psm = ck1.tile([C, C], bf16, tag="psm")
                        nc.vector.tensor_mul(psm[:], psP[:], mask_ut)
                        vs = slice(h * V, (h + 1) * V)
                        nc.tensor.matmul(
                            oP[:, vs], ptT[h * M:(h + 1) * M, :],
                            hv[h * M:(h + 1) * M, :], start=True, stop=False)
                        nc.tensor.matmul(
                            oP[:, vs], psm[:],
                            vn[:, i * KP + h * V:i * KP + (h + 1) * V],
                            start=False, stop=True)
                    # state update: U then scan step
                    uP = psbig.tile([128, 512], f32, tag="big")
                    for h in range(HP):
                        hp = slice(h * M, (h + 1) * M)
                        nc.tensor.matmul(uP[h * M:(h + 1) * M, 0:256],
                                         sa[:, hp],
                                         knat[:, h * 256:(h + 1) * 256])
                        nc.tensor.matmul(uP[h * M:(h + 1) * M, 256:512],
                                         sa[:, hp],
                                         vn[:, i * KP + h * V:
                                            i * KP + (h + 1) * V])
                    dec = ck1.tile([128, 512], f32, tag="dec")
                    nc.vector.tensor_scalar_mul(dec[:], state[:], atc[:])
                    state_n = stp.tile([128, 512], f32, tag="state")
                    nc.vector.tensor_add(state_n[:], dec[:], uP[:])
                    state = state_n
                    # RMSNorm from PSUM, write o_bf, transpose to oT
                    obf = ck1.tile([C, 512], bf16, tag="obf")
                    for h in range(HP):
                        vs = slice(h * V, (h + 1) * V)
                        sq = ck1.tile([C, V], bf16, tag="sq")
                        ssq = ckp.tile([C, 1], f32, tag="ssq")
                        nc.scalar.activation(sq[:], oP[:, vs], AF.Square,
                                             accum_out=ssq[:])
                        rms = ckp.tile([C, 1], f32, tag="rms")
                        nc.scalar.activation(rms[:], ssq[:], AF.Sqrt,
                                             bias=epsc[:], scale=1.0 / V)
                        rrms = ckp.tile([C, 1], f32, tag="rrms")
                        nc.vector.reciprocal(rrms[:], rms[:])
                        nc.vector.tensor_scalar_mul(obf[:, vs], oP[:, vs],
                                                    rrms[:])
                    for cc in range(4):
                        oTP = pst.tile([128, C], bf16, tag="tp")
                        nc.tensor.transpose(
                            oTP[:], obf[:, cc * 128:(cc + 1) * 128], eye_bf)
                        nc.any.tensor_copy(oT[:, cc * T + t0:cc * T + t0 + C],
                                           oTP[:])
                    # y tile for this chunk (only depends on this chunk's oT)
                    ysb = ck1.tile([128, D], bf16, tag="ysb")
                    for half in range(2):
                        yps = psgr.tile([128, 512], f32, tag="gram")
                        for kt in range(4):
                            nc.tensor.matmul(
                                yps[:], oT[:, kt * T + t0:kt * T + t0 + 128],
                                wo[:, kt * D + half * 512:
                                   kt * D + (half + 1) * 512],
                                start=(kt == 0), stop=(kt == 3))
                        nc.any.tensor_copy(
                            ysb[:, half * 512:(half + 1) * 512], yps[:])
                    nc.sync.dma_start(yb[t0:t0 + 128, :], ysb[:])
                if SIM:
                    nc.gpsimd.dma_start(ybh[:], yb[0:T // 2, :])
                else:
                    nc.gpsimd.collective_compute(
                        "ReduceScatter", mybir.AluOpType.add,
                        replica_groups=[[0, 1], [2, 3], [4, 5], [6, 7]],
                        ins=[yb.opt()], outs=[ybh.opt()])
                nc.gpsimd.dma_start(y_d[:], ybh[:])

    nc.compile()
    return nc


def _get_program():
    if "nc" not in _cache:
        _cache["nc"] = _build_program()
    return _cache["nc"]


def _get_runner():
    """jit-once shard_map runner over 8 cores, modeled on
    bass2jax.run_bass_via_pjrt but reusable with device-staged inputs.
    Output-seed zero buffers are created once on device and reused (no
    donation; the kernel writes every output element)."""
    if "runner" in _cache:
        return _cache["runner"]
    import jax
    import jax.numpy as jnp
    from jax.sharding import Mesh, PartitionSpec, NamedSharding
    from jax.experimental.shard_map import shard_map
    from concourse import bass2jax, mybir

    nc = _get_program()
    bass2jax.install_neuronx_cc_hook()
    pname = nc.partition_id_tensor.name if nc.partition_id_tensor else None
    in_names, out_names, out_avals = [], [], []
    for alloc in nc.m.functions[0].allocations:
        if not isinstance(alloc, mybir.MemoryLocationSet):
            continue
        name = alloc.memorylocations[0].name
        if alloc.kind == "ExternalInput":
            if name != pname:
                in_names.append(name)
        elif alloc.kind == "ExternalOutput":
            out_names.append(name)
            out_avals.append(jax.core.ShapedArray(
                tuple(alloc.tensor_shape), mybir.dt.np(alloc.dtype)))
    n_params = len(in_names)
    all_names = in_names + out_names
    if pname is not None:
        all_names = all_names + [pname]
    nio = n_params + len(out_names)

    def _body(*args):
        operands = list(args)
        if pname is not None:
            operands.append(bass2jax.partition_id_tensor())
        return tuple(bass2jax._bass_exec_p.bind(
            *operands,
            out_avals=tuple(out_avals),
            in_names=tuple(all_names),
            out_names=tuple(out_names),
            lowering_input_output_aliases=(),
            sim_require_finite=True,
            sim_require_nnan=True,
            nc=nc,
        ))

    assert tuple(in_names) == IN_ORDER, in_names
    spec = _get_spec()
    mesh = spec.mesh
    sharded = jax.jit(
        shard_map(_body, mesh=mesh,
                  in_specs=(PartitionSpec("core"),) * nio,
                  out_specs=(PartitionSpec("core"),) * len(out_names),
                  check_rep=False),
        keep_unused=True)
    _cache["runner"] = (sharded, in_names, out_names, out_avals, spec)
    return _cache["runner"]


def _get_spec():
    if "spec" not in _cache:
        import jax
        from jax.sharding import Mesh, PartitionSpec, NamedSharding
        mesh = Mesh(np.asarray(jax.devices()[:8]), ("core",))
        _cache["spec"] = NamedSharding(mesh, PartitionSpec("core"))
    return _cache["spec"]


def _stage_inputs(concat):
    # Pure RPC work (device_put + on-device zeros); the caller pre-builds
    # the concatenated host arrays so this thread mostly waits GIL-free and
    # overlaps the bass program build on the main thread.
    import jax
    import jax.numpy as jnp
    import ml_dtypes
    from concurrent.futures import ThreadPoolExecutor
    spec = _get_spec()
    with ThreadPoolExecutor(len(concat)) as ex:
        dev_in = list(ex.map(lambda a: jax.device_put(a, spec), concat))
    if "dev_zeros" not in _cache:
        zeros_fn = jax.jit(
            lambda: tuple(jnp.zeros((8 * s[0],) + tuple(s[1:]),
                                    ml_dtypes.bfloat16)
                          for s in OUT_SHAPES),
            out_shardings=tuple(spec for _ in OUT_SHAPES))
        _cache["dev_zeros"] = zeros_fn()
    jax.block_until_ready(dev_in)
    jax.block_until_ready(_cache["dev_zeros"])
    return dev_in


def _exec(dev_in):
    import jax
    sharded = _get_runner()[0]
    outs = sharded(*dev_in, *_cache["dev_zeros"])
    jax.block_until_ready(outs)
    return outs


def _fetch(arr, dtype=np.float32):
    """Pull a sharded device array to host with one RPC stream per shard."""
    from concurrent.futures import ThreadPoolExecutor
    shards = sorted(arr.addressable_shards,
                    key=lambda s: s.index[0].start or 0)
    with ThreadPoolExecutor(len(shards)) as ex:
        parts = list(ex.map(lambda s: np.asarray(s.data, dtype), shards))
    return np.stack(parts)


def benchmark(iters=4, depth=24):
    """Estimate per-execution device time. A single blocked execution is
    dominated by the ~50-80 ms axon RPC dispatch floor, so additionally
    dispatch `depth` executions asynchronously (they queue on the devices)
    and block once; the marginal wall per added execution is the steady-
    state per-execution time with dispatch overhead amortized. Returns the
    marginal estimate when it is coherent, else the best single-exec wall.
    Needs a prior kernel() call."""
    import time
    import jax
    dev_in = _cache["dev_in"]
    sharded = _get_runner()[0]
    zs = _cache["dev_zeros"]
    _exec(dev_in)  # warm dispatch path

    best1 = None
    for _ in range(iters):
        t0 = time.perf_counter_ns()
        _exec(dev_in)
        t1 = time.perf_counter_ns()
        best1 = t1 - t0 if best1 is None else min(best1, t1 - t0)
    bestd = None
    for _ in range(iters):
        t0 = time.perf_counter_ns()
        outs = [sharded(*dev_in, *zs) for _ in range(depth)]
        jax.block_until_ready(outs)
        t1 = time.perf_counter_ns()
        bestd = t1 - t0 if bestd is None else min(bestd, t1 - t0)
    marginal = (bestd - best1) // (depth - 1)
    if 0 < marginal < best1:
        return marginal
    return best1


def kernel(x, Wq, Wk, Wv, Wf, g_norm_w, Wo):
    import ml_dtypes
    bf = ml_dtypes.bfloat16

    x = np.asarray(x, np.float32)
    Wq = np.asarray(Wq, np.float32)
    Wk = np.asarray(Wk, np.float32)
    Wv = np.asarray(Wv, np.float32)
    Wf = np.asarray(Wf, np.float32)
    gw = np.asarray(g_norm_w, np.float32)
    Wo = np.asarray(Wo, np.float32)

    consts = _build_consts()
    gw_full = np.tile(gw, HP)                       # [512]
    bundles = []
    for hp in range(HP):
        sk = slice(hp * KP, (hp + 1) * KP)
        sf = slice(hp * MP, (hp + 1) * MP)
        wo_s = Wo[sk, :] * gw_full[:, None]
        bundles.append(np.concatenate(
            [Wq[:, sk].ravel(), Wk[:, sk].ravel(), Wv[:, sk].ravel(),
             Wf[:, sf].ravel(), wo_s.ravel()]).astype(bf))
    xT = [x[b].T.astype(bf) for b in range(B)]      # [D, T] bf16 per batch
    in_maps = []
    for core in range(8):
        b, hp = core // 2, core % 2
        in_maps.append({
            "xh": np.ascontiguousarray(
                xT[b][:, hp * (T // 2):(hp + 1) * (T // 2)]),
            "wb": bundles[hp][b * WBQ:(b + 1) * WBQ],
            "cn": consts,
        })

    import threading
    concat = [np.concatenate([in_maps[c][n] for c in range(8)], axis=0)
              for n in IN_ORDER]
    staged = {}

    def _put():
        staged["dev_in"] = _stage_inputs(concat)

    th = threading.Thread(target=_put)
    th.start()
    _get_runner()          # build bass program (overlaps the transfers)
    th.join()
    dev_in = staged["dev_in"]
    _cache["dev_in"] = dev_in
    outs = _exec(dev_in)
    _, _, out_names, out_avals, _ = _get_runner()
    yi = out_names.index("y")
    y_half = _fetch(outs[yi]).reshape(8, T // 2, D)
    y = np.empty((B, T, D), np.float32)
    for b in range(B):
        y[b, 0:T // 2] = y_half[2 * b]
        y[b, T // 2:T] = y_half[2 * b + 1]
    return y

